# revision 67
# baseline (speedup 1.0000x reference)
"""Trainium2 Bass kernel for nn_AttentionBlock (B=2, T=2048, D=1024, H=16,
Dh=64, Ff=4096), SPMD across 8 NeuronCores in one NEFF launch.

Sharding:
  - Phase 1+2 (QKV projection + attention): 2 heads per core, fp8.
  - AllToAll (1 MiB/core, bf16) re-shards attention output heads->tokens.
  - Phase 3 (out-proj + residual + LayerNorm + MLP): 512 tokens/core, bf16.

Numerics / speed tricks:
  - QKV projection, scores (Q.K^T), and attn@V all run as fp8e4m3
    DoubleRow matmuls (256-wide contraction pairs).
  - Alibi is delivered fp8 and added to scores IN PSUM by an extra
    DoubleRow matmul against a block-identity (PE "inject"), so the
    elementwise engines see a single fused (s+a) tile.
  - exp() of the 256 score tiles is split across three engines: ACT does
    real Exp -> fp8e5m2; DVE and GPSIMD compute the e5m2 BIT PATTERN
    directly as u8 = floor(x*4*log2e + 60.29) (max rel err ~11%, same as
    e5m2 quantization itself; softmax normalization cancels most of it).
  - Softmax denominator comes from a ones-column appended to V; the
    1/denom broadcast across partitions runs on the PE (ones x rec).
  - Phase 3 stays bf16 (fp8 noise summed over the 4096-dim MLP
    contraction would breach the 2e-2 gate); b_mlp_out is injected into
    the out-proj PSUM via a ones-row matmul to save a residual reload.

kernel(**inputs) takes FULL unsharded inputs, returns the FULL output.
"""

import sys

for _p in ("/opt/trn_rl_repo", "/root/.axon_site/_ro/trn_rl_repo"):
    if _p not in sys.path:
        sys.path.insert(0, _p)

import numpy as np
import ml_dtypes

import concourse.bass as bass
import concourse.tile as tile
from concourse import bacc, mybir
from concourse.bass_utils import run_bass_kernel_spmd
from concourse.masks import make_identity

BF16 = ml_dtypes.bfloat16
E4NP = ml_dtypes.float8_e4m3
E5NP = ml_dtypes.float8_e5m2

B, T, D, H, Dh, FF = 2, 2048, 1024, 16, 64, 4096
NTOK = B * T            # 4096
NCORES = 8
CHUNK = NTOK // NCORES  # 512 tokens per core
HPC = H // NCORES       # 2 heads per core
KT = T // 128           # 16 k-tiles per batch

F32 = mybir.dt.float32
BF = mybir.dt.bfloat16
E4 = mybir.dt.float8e4
E5 = mybir.dt.float8e5
U8 = mybir.dt.uint8
AF = mybir.ActivationFunctionType
DR = mybir.MatmulPerfMode.DoubleRow
ALU = mybir.AluOpType

LOG2E = 1.4426950408889634
SP_SCALE = 512.0           # PSUM holds 512*(s+alibi): w x8, alibi x256 x ident 2
EXP_C1 = 8.0 * LOG2E / SP_SCALE  # e4m3 bits-exp scale
EXP_C2 = 56.13             # e4m3 bits-exp bias (calibrated for trunc)

# exp engine schedule per kt-pair: ACT (real Exp) vs DVE (bits-exp).
# gpsimd cannot read PSUM, so Pool only gets the SBUF->SBUF broadcast.
EXP_PAT = "ADAAD"

_COMPILED = None


def _build(sim1=False):
    nc = bacc.Bacc("TRN2", target_bir_lowering=False, debug=False,
                   num_devices=1 if sim1 else NCORES)

    # ---- kernel I/O (per core) ----
    # xT2[kk, p, i, tok] = x[tok, kk*256 + i*128 + p]  (fp8)
    xT2_io = nc.dram_tensor("xT2", [4, 128, 2, NTOK], E4, kind="ExternalInput").ap()
    # wq2[kk, p, i, m]; m 0:128 q (i_q*64 + hl*32 + po), 128:256 k, 256:384 v
    wq2_io = nc.dram_tensor("wq2", [4, 128, 2, 384], E4, kind="ExternalInput").ap()
    # a2[hl, p, kt, i, q] = alibi^T[hl, k=kt*128+i*64+p, q]  (fp8)
    a2_io = nc.dram_tensor("a2", [HPC, 64, KT, 2, T], E4, kind="ExternalInput").ap()
    w_outT_io = nc.dram_tensor("w_outT", [D, D], BF, kind="ExternalInput").ap()
    b_outT_io = nc.dram_tensor("b_outT", [1, D], BF, kind="ExternalInput").ap()
    x_res_io = nc.dram_tensor("x_res", [CHUNK, D], F32, kind="ExternalInput").ap()
    # packed as [p, ff, kk, fin] = w_mlp_in_eff[ff*128+fin, kk*128+p]
    # hi/lo e5m2 split: w ~ wa + wb with |wb| ~ 7% |w|
    w_inP_io = nc.dram_tensor("w_inP", [128, 32, 8, 128], E5, kind="ExternalInput").ap()
    w_inPb_io = nc.dram_tensor("w_inPb", [128, 32, 8, 128], E5, kind="ExternalInput").ap()
    b_inT_io = nc.dram_tensor("b_inT", [128, 32], F32, kind="ExternalInput").ap()
    w_mlp_outT_io = nc.dram_tensor("w_mlp_outT", [FF, D], BF, kind="ExternalInput").ap()
    out_io = nc.dram_tensor("out", [CHUNK, D], F32, kind="ExternalOutput").ap()
    import os as _os
    DBG = _os.environ.get("KDBG") == "1"
    if DBG:
        dbg_q_io = nc.dram_tensor("dbg_q", [64, 2, T], U8, kind="ExternalOutput").ap()
        dbg_k_io = nc.dram_tensor("dbg_k", [64, 2, T], U8, kind="ExternalOutput").ap()
        dbg_v_io = nc.dram_tensor("dbg_v", [128, 8, 2, 2, 72], U8, kind="ExternalOutput").ap()
        dbg_sp_io = nc.dram_tensor("dbg_sp", [128, 2, 512], F32, kind="ExternalOutput").ap()
        dbg_ex_io = nc.dram_tensor("dbg_ex", [128, 2, 512], U8, kind="ExternalOutput").ap()
        dbg_yn_io = nc.dram_tensor("dbg_yn", [64, 512], BF, kind="ExternalOutput").ap()

    # ---- internal DRAM ----
    # two half-alltoalls (one per local head): piece 0 fires at the
    # attention midpoint so half the out-proj input lands early
    cc_send0 = nc.dram_tensor("cc_send0", [NCORES * 64, CHUNK], BF)
    cc_recv0 = nc.dram_tensor("cc_recv0", [NCORES * 64, CHUNK], BF)
    cc_send1 = nc.dram_tensor("cc_send1", [NCORES * 64, CHUNK], BF)
    cc_recv1 = nc.dram_tensor("cc_recv1", [NCORES * 64, CHUNK], BF)

    with tile.TileContext(nc) as tc:
        with tc.tile_pool(name="consts", bufs=1) as consts:
            identb = consts.tile([128, 128], BF, tag="identb")
            make_identity(nc, identb[:])
            # inject identity: I2[p, i, m] = 1 iff m == i*64 + p
            i2 = consts.tile([64, 2, 128], E4, tag="i2")
            nc.gpsimd.memset(i2[:], 0.0)
            nc.gpsimd.affine_select(
                out=i2[:, 0, :], in_=i2[:, 0, :],
                compare_op=ALU.not_equal, fill=2.0,
                base=0, pattern=[[-1, 128]], channel_multiplier=1)
            nc.gpsimd.affine_select(
                out=i2[:, 1, :], in_=i2[:, 1, :],
                compare_op=ALU.not_equal, fill=2.0,
                base=64, pattern=[[-1, 128]], channel_multiplier=1)
            ones1 = consts.tile([1, 128], BF, tag="ones1")
            nc.vector.memset(ones1[:], 1.0)
            warm_f = consts.tile([128, 512], F32, tag="warm_f")
            nc.vector.memset(warm_f[:], 0.5)
            warm_rhs = consts.tile([128, 512], BF, tag="warm_rhs")
            nc.vector.tensor_copy(warm_rhs[:], warm_f[:])

            with tc.tile_pool(name="qkv", bufs=1) as qkv:
                # q/k packed for DoubleRow scores: [hl*32+p, i, tok],
                # value = q[head hl, d=i*32+p, tok]
                qT2s, kT2s, vs = [], [], []
                for b in range(2):
                    qT2 = qkv.tile([64, 2, T], E4, tag=f"qT2{b}", name=f"qT2{b}")
                    kT2 = qkv.tile([64, 2, T], E4, tag=f"kT2{b}", name=f"kT2{b}")
                    # v[p, pr, j, hl, dh]: k-token kt=2*pr+j block, ones at dh=64
                    vb = qkv.tile([128, 8, 2, 2, 72], E4, tag=f"v{b}",
                                  name=f"v{b}")
                    nc.vector.memset(vb[:, :, :, :, 64:65], 8.0)
                    qT2s.append(qT2); kT2s.append(kT2); vs.append(vb)

                albp = tc.alloc_tile_pool(name="alb", bufs=4)
                als = {}

                def al_prefetch(hl, qc):
                    if (hl, qc) in als or hl >= 2:
                        return
                    al = albp.tile([64, KT, 2, 512], E4, tag="al",
                                   name=f"al{hl}_{qc}")
                    nc.sync.dma_start(
                        al[:], a2_io[hl, :, :, :, qc * 512:(qc + 1) * 512])
                    als[(hl, qc)] = al

                with tc.tile_pool(name="p1x", bufs=1) as p1x, \
                     tc.tile_pool(name="p1w", bufs=1) as p1w, \
                     tc.tile_pool(name="p1ps", bufs=2, space="PSUM") as p1ps, \
                     tc.tile_pool(name="p1t", bufs=3) as p1t, \
                     tc.tile_pool(name="p1pt", bufs=1, space="PSUM") as p1pt:
                    wq = []
                    for kk in range(4):
                        w = p1w.tile([128, 2, 384], E4, tag=f"wq{kk}")
                        nc.sync.dma_start(w[:], wq2_io[kk])
                        wq.append(w)

                    def proj_pass(b):
                        qT2, kT2, v_all = qT2s[b], kT2s[b], vs[b]
                        with nc.named_scope(f"qkvproj{b}"):
                            xts = [p1x.tile([128, 2, T], E4, tag=f"xt{kk}",
                                            name=f"xt{kk}_{b}")
                                   for kk in range(4)]
                            for qt in range(4):
                                for kk in range(4):
                                    nc.sync.dma_start(
                                        xts[kk][:, :, qt * 512:(qt + 1) * 512],
                                        xT2_io[kk, :, :,
                                               b * T + qt * 512:
                                               b * T + (qt + 1) * 512])
                            for t in range(4):
                                ts = slice(t * 512, (t + 1) * 512)
                                # q/k: two matmul groups each (i pair fold)
                                for name, mo, dst in (("q", 0, qT2),
                                                      ("k", 128, kT2)):
                                    ps = p1ps.tile([128, 2, 512], F32,
                                                   tag="pq",
                                                   name=f"ps{name}_{b}_{t}")
                                    for i in range(2):
                                        for c in range(2):
                                            cs = slice(c * 256, (c + 1) * 256)
                                            rs = slice(t * 512 + c * 256,
                                                       t * 512 + (c + 1) * 256)
                                            for kk in range(4):
                                                nc.tensor.matmul(
                                                    ps[0:64, i, cs],
                                                    wq[kk][:, :, mo + i * 64:
                                                           mo + (i + 1) * 64],
                                                    xts[kk][:, :, rs],
                                                    start=(kk == 0),
                                                    stop=(kk == 3),
                                                    perf_mode=DR)
                                    if name == "q":
                                        nc.vector.tensor_copy(
                                            dst[:, :, ts], ps[0:64, :, :])
                                    else:
                                        nc.scalar.activation(
                                            dst[:, :, ts], ps[0:64, :, :],
                                            AF.Copy)
                                # v
                                psv = p1ps.tile([128, 512], F32, tag="pp",
                                                name=f"psv_{b}_{t}")
                                for c in range(2):
                                    cs = slice(c * 256, (c + 1) * 256)
                                    rs = slice(t * 512 + c * 256,
                                               t * 512 + (c + 1) * 256)
                                    for kk in range(4):
                                        nc.tensor.matmul(
                                            psv[:, cs],
                                            wq[kk][:, :, 256:384],
                                            xts[kk][:, :, rs],
                                            start=(kk == 0), stop=(kk == 3),
                                            perf_mode=DR)
                                vt = p1t.tile([128, 512], BF, tag="vt",
                                              name=f"vt{b}_{t}")
                                nc.vector.tensor_copy(vt[:], psv[:])
                                for tc_ in range(4):
                                    ti = t * 4 + tc_
                                    pt = p1pt.tile([128, 128], BF, tag="pt",
                                                   name=f"pt{b}_{ti}")
                                    nc.tensor.transpose(
                                        pt[:], vt[:, tc_ * 128:(tc_ + 1) * 128],
                                        identb[:])
                                    nc.vector.tensor_copy(
                                        v_all[:, ti // 2, ti % 2, :, 0:64],
                                        pt[:].rearrange("p (a b) -> p a b", a=2))

                    for wi_ in range(14):
                        wps = p1ps.tile([128, 512], F32, tag="pp",
                                        name=f"warms{wi_}")
                        nc.tensor.matmul(wps[:], identb[:], warm_rhs[:],
                                         start=True, stop=True)
                    proj_pass(0)
                    al_prefetch(0, 0)
                    proj_pass(1)
                    al_prefetch(0, 1)

                with nc.named_scope("attn"), \
                     tc.tile_pool(name="exps", bufs=14) as expp, \
                     tc.tile_pool(name="sps", bufs=6, space="PSUM") as spsp, \
                     tc.tile_pool(name="yups", bufs=2, space="PSUM") as yupp, \
                     tc.tile_pool(name="nrm", bufs=6) as nrmp:
                    exp_n = 0

                    def attn_pass(hl, qc, b, al):
                        nonlocal exp_n
                        ctx = nc.allow_low_precision(
                            reason="fp8 softmax path; normalization is "
                                   "relative so bf16/e5m2 noise is benign")
                        ctx.__enter__()
                        yu = yupp.tile([65, 512], F32, tag="yu",
                                       name=f"yu{hl}_{qc}_{b}")
                        ex2s = []
                        for pr in range(8):
                            ex2 = expp.tile([128, 2, 512], E4, tag="ex",
                                            name=f"ex_{hl}_{qc}_{b}_{pr}")
                            ex2s.append(ex2)
                            for j in range(2):
                                kt = pr * 2 + j
                                sp = spsp.tile([128, 512], F32, tag="sp",
                                               name=f"sp{hl}_{qc}_{b}_{kt}")
                                for c in range(2):
                                    cs = slice(c * 256, (c + 1) * 256)
                                    qs = slice(qc * 512 + c * 256,
                                               qc * 512 + (c + 1) * 256)
                                    nc.tensor.matmul(
                                        sp[:, cs],
                                        kT2s[b][hl * 32:(hl + 1) * 32, :,
                                                kt * 128:(kt + 1) * 128],
                                        qT2s[b][hl * 32:(hl + 1) * 32, :, qs],
                                        start=True, stop=False, perf_mode=DR)
                                    nc.tensor.matmul(
                                        sp[:, cs], i2[:],
                                        al[:, kt, :, cs],
                                        start=False, stop=True, perf_mode=DR)
                                eng = EXP_PAT[exp_n % len(EXP_PAT)]
                                exp_n += 1
                                if eng == "A":
                                    nc.scalar.activation(ex2[:, j, :], sp[:],
                                                         AF.Exp,
                                                         scale=1.0 / SP_SCALE)
                                else:
                                    nc.vector.tensor_scalar(
                                        ex2[:, j, :].bitcast(U8), sp[:],
                                        EXP_C1, EXP_C2, ALU.mult, ALU.add)
                            if DBG and (hl, qc, b, pr) == (0, 0, 0, 0):
                                spd = nrmp.tile([128, 2, 512], F32, tag="spd")
                                nc.vector.tensor_copy(spd[:], sp[:])
                                nc.sync.dma_start(dbg_sp_io, spd[:])
                                nc.sync.dma_start(dbg_ex_io, ex2[:].bitcast(U8))
                        for c in range(2):
                            cs = slice(c * 256, (c + 1) * 256)
                            for pr in range(8):
                                nc.tensor.matmul(
                                    yu[:, cs],
                                    vs[b][:, pr, :, hl, 0:65],
                                    ex2s[pr][:, :, cs],
                                    start=(pr == 0), stop=(pr == 7),
                                    perf_mode=DR)
                        rec = nrmp.tile([1, 512], BF, tag="rec",
                                        name=f"rec{hl}_{qc}_{b}")
                        nc.vector.reciprocal(rec[:], yu[64:65, :])
                        bc = nrmp.tile([64, 512], BF, tag="bc",
                                       name=f"bc{hl}_{qc}_{b}")
                        nc.gpsimd.partition_broadcast(bc[:], rec[:])
                        yn = nrmp.tile([64, 512], BF, tag="yn",
                                       name=f"yn{hl}_{qc}_{b}")
                        nc.vector.tensor_mul(yn[:], yu[0:64, :], bc[:])
                        cc_s = cc_send0 if hl == 0 else cc_send1
                        row = (b * 4 + qc) * 64
                        nc.sync.dma_start(
                            bass.AP(tensor=cc_s, offset=row * 512,
                                    ap=[[512, 64], [1, 512]]),
                            yn[:])
                        if DBG and (hl, qc, b) == (0, 0, 0):
                            nc.sync.dma_start(dbg_yn_io, yn[:])
                        ctx.__exit__(None, None, None)

                    for hl in range(2):
                        for qc in range(4):
                            al_prefetch(hl, qc)
                            nxt = hl * 4 + qc + 1
                            al_prefetch(nxt // 4, nxt % 4)
                            for b in range(2):
                                attn_pass(hl, qc, b, als[(hl, qc)])
                        with nc.named_scope(f"a2a{hl}"):
                            cs_, cr_ = ((cc_send0, cc_recv0) if hl == 0
                                        else (cc_send1, cc_recv1))
                            if sim1:
                                nc.sync.dma_start(cr_[:], cs_[:])
                            else:
                                nc.gpsimd.collective_compute(
                                    "AllToAll", mybir.AluOpType.bypass,
                                    replica_groups=[list(range(NCORES))],
                                    ins=[cs_[:]], outs=[cr_[:]])

                if DBG:
                    nc.sync.dma_start(dbg_q_io, qT2s[0][:].bitcast(U8))
                    nc.sync.dma_start(dbg_k_io, kT2s[0][:].bitcast(U8))
                    nc.sync.dma_start(dbg_v_io, vs[0][:].bitcast(U8))
                albp.release()

            # ---------------- phase 3: out-proj + LN + MLP ----------------
            with nc.named_scope("mlp"), \
                 tc.tile_pool(name="p3w", bufs=1) as p3w, \
                 tc.tile_pool(name="p3acc", bufs=2, space="PSUM") as p3acc, \
                 tc.tile_pool(name="p3mo", bufs=4, space="PSUM") as p3mo, \
                 tc.tile_pool(name="p3pt", bufs=2, space="PSUM") as p3pt, \
                 tc.tile_pool(name="p3sb", bufs=1) as p3sb, \
                 tc.tile_pool(name="p3r", bufs=3) as p3r, \
                 tc.tile_pool(name="p3s", bufs=4) as p3s, \
                 tc.tile_pool(name="mlpw", bufs=8) as mlpw:
                for wi_ in range(60):
                    wps = p3pt.tile([128, 512], F32, tag="pt3",
                                    name=f"warm{wi_}")
                    nc.tensor.matmul(wps[:], identb[:], warm_rhs[:],
                                     start=True, stop=True)
                yrT = p3w.tile([128, 8, 512], BF, tag="yrT")
                nc.scalar.dma_start(
                    yrT[0:64, :, :],
                    bass.AP(tensor=cc_recv0, offset=0,
                            ap=[[512, 64], [64 * 512, 8], [1, 512]]))
                nc.scalar.dma_start(
                    yrT[64:128, :, :],
                    bass.AP(tensor=cc_recv1, offset=0,
                            ap=[[512, 64], [64 * 512, 8], [1, 512]]))
                yrecv = [yrT[:, kk, :] for kk in range(8)]
                wout = []
                for kk in range(8):
                    wo = p3w.tile([128, D], BF, tag=f"wo{kk}")
                    nc.sync.dma_start(wo[:], w_outT_io[kk * 128:(kk + 1) * 128, :])
                    wout.append(wo)
                b_out = p3w.tile([1, D], BF, tag="b_out")
                nc.sync.dma_start(b_out[:], b_outT_io)
                b_in = p3sb.tile([128, 32], F32, tag="b_in")
                nc.sync.dma_start(b_in[:], b_inT_io)

                y_sb = p3sb.tile([128, 4, D], F32, tag="y_sb")
                hT = p3sb.tile([128, 8, 512], E5, tag="hT")
                hTb = p3sb.tile([128, 8, 512], E5, tag="hTb")
                lp3 = nc.allow_low_precision(
                    reason="hi-lo e5m2 split of LN output; residual carries "
                           "the low bits so total error < bf16")
                lp3.__enter__()
                x_res_r = x_res_io.rearrange("(t p) d -> p t d", p=128)
                for tt in range(4):
                    xr = p3r.tile([128, D], F32, tag="xr")
                    nc.sync.dma_start(xr[:], x_res_r[:, tt, :])
                    for dc in range(2):
                        ds = slice(dc * 512, (dc + 1) * 512)
                        ps = p3acc.tile([128, 512], F32, tag="acc")
                        for kk in range(8):
                            nc.tensor.matmul(
                                ps[:], yrecv[kk][:, tt * 128:(tt + 1) * 128],
                                wout[kk][:, ds],
                                start=(kk == 0), stop=(kk == 7))
                        nc.vector.tensor_add(
                            y_sb[:, tt, ds], ps[:], xr[:, ds])
                    # LayerNorm + transpose for this tt, overlapping the
                    # next tt's out-proj matmuls
                    stats = p3s.tile([128, 2, 6], F32, tag="stats")
                    for g in range(2):
                        nc.vector.bn_stats(
                            stats[:, g, :],
                            y_sb[:, tt, g * 512:(g + 1) * 512])
                    mv = p3s.tile([128, 2], F32, tag="mv")
                    nc.vector.bn_aggr(mv[:], stats[:])
                    eps = p3s.tile([128, 1], F32, tag="eps")
                    nc.vector.memset(eps[:], 1e-5)
                    sd = p3s.tile([128, 1], F32, tag="sd")
                    nc.scalar.activation(sd[:], mv[:, 1:2], AF.Sqrt,
                                         bias=eps[:], scale=1.0)
                    rstd = p3s.tile([128, 1], F32, tag="rstd")
                    nc.vector.reciprocal(rstd[:], sd[:])
                    nb = p3s.tile([128, 1], F32, tag="nb")
                    nc.vector.tensor_mul(nb[:], mv[:, 0:1], rstd[:])
                    nb2 = p3s.tile([128, 1], F32, tag="nb2")
                    nc.scalar.mul(nb2[:], nb[:], -1.0)
                    hn = p3r.tile([128, D], BF, tag="hn")
                    nc.scalar.activation(hn[:], y_sb[:, tt, :], AF.Identity,
                                         bias=nb2[:], scale=rstd[:])
                    for dc in range(8):
                        pt = p3pt.tile([128, 128], BF, tag="pt3")
                        nc.tensor.transpose(
                            pt[:], hn[:, dc * 128:(dc + 1) * 128], identb[:])
                        hs = (slice(None), dc, slice(tt * 128, (tt + 1) * 128))
                        nc.scalar.activation(hT[hs], pt[:], AF.Copy)
                        nc.vector.tensor_sub(hTb[hs], pt[:], hT[hs])

                # MLP in + gelu -> hmT (Ff-major bf16)
                hmT = p3sb.tile([128, 32, 512], BF, tag="hmT")
                for ff in range(32):
                    wi = mlpw.tile([128, 8, 128], E5, tag="wi")
                    nc.sync.dma_start(wi[:], w_inP_io[:, ff, :, :])
                    wib = mlpw.tile([128, 8, 128], E5, tag="wib")
                    nc.sync.dma_start(wib[:], w_inPb_io[:, ff, :, :])
                    ps = p3acc.tile([128, 512], F32, tag="acc")
                    for c_ in range(2):
                        cs_ = slice(c_ * 256, (c_ + 1) * 256)
                        nmm = 0
                        for lhs, rhs in ((wi, hT), (wi, hTb), (wib, hT)):
                            for pr_ in range(4):
                                nc.tensor.matmul(
                                    ps[:, cs_],
                                    lhs[:, pr_ * 2:pr_ * 2 + 2, :],
                                    rhs[:, pr_ * 2:pr_ * 2 + 2, cs_],
                                    start=(nmm == 0), stop=(nmm == 11),
                                    perf_mode=DR)
                                nmm += 1
                    nc.scalar.activation(hmT[:, ff, :], ps[:], AF.Gelu,
                                         bias=b_in[:, ff:ff + 1], scale=1.0)
                lp3.__exit__(None, None, None)

                # MLP out + final residual
                out_r = out_io.rearrange("(t p) d -> p t d", p=128)
                for dc in range(2):
                    ds = slice(dc * 512, (dc + 1) * 512)
                    pss = [p3mo.tile([128, 512], F32, tag="mo",
                                     name=f"mo{dc}_{i}") for i in range(4)]
                    last = mlpw.tile([128, 512], BF, tag="wo2",
                                     name=f"wo2last{dc}")
                    for ff in range(32):
                        if ff < 31:
                            wo2 = mlpw.tile([128, 512], BF, tag="wo2")
                        else:
                            wo2 = last
                        nc.sync.dma_start(
                            wo2[:], w_mlp_outT_io[ff * 128:(ff + 1) * 128, ds])
                        if ff == 31:
                            break
                        for tt in range(4):
                            nc.tensor.matmul(
                                pss[tt][:],
                                hmT[:, ff, tt * 128:(tt + 1) * 128], wo2[:],
                                start=(ff == 0), stop=False)
                    for tt in range(4):
                        # last ff slice + bias, then drain this tt while
                        # the others still accumulate
                        nc.tensor.matmul(
                            pss[tt][:],
                            hmT[:, 31, tt * 128:(tt + 1) * 128], last[:],
                            start=False, stop=False)
                        # + b_mlp_out broadcast via ones-row matmul
                        nc.tensor.matmul(
                            pss[tt][:], ones1[0:1, 0:128], b_out[:, ds],
                            start=False, stop=True)
                        fin = p3s.tile([128, 512], F32, tag="fin")
                        nc.vector.tensor_add(
                            fin[:], pss[tt][:],
                            y_sb[:, tt, ds])
                        nc.sync.dma_start(
                            out_r[:, tt, ds], fin[:])

    nc.compile()
    return nc


def _host_prep(x, alibi, ln1_w, w_qkv, w_out, ln2_w, w_mlp_in, b_mlp_in,
               w_mlp_out, b_mlp_out):
    f32 = np.float32
    x = np.asarray(x, f32)
    x_flat = np.ascontiguousarray(x.reshape(NTOK, D))
    w_qkv = np.asarray(w_qkv, f32)
    w_out = np.asarray(w_out, f32)
    w_mlp_in = np.asarray(w_mlp_in, f32)
    w_mlp_out = np.asarray(w_mlp_out, f32)
    b_mlp_in = np.asarray(b_mlp_in, f32)
    b_mlp_out = np.asarray(b_mlp_out, f32)
    ln2_w = np.asarray(ln2_w, f32)
    alibi = np.asarray(alibi, f32)

    # xT2[kk, p, i, tok] = x[tok, kk*256 + i*128 + p]
    xT2 = np.ascontiguousarray(
        x_flat.T.reshape(4, 2, 128, NTOK).transpose(0, 2, 1, 3)).astype(E4NP)

    w_outT = np.ascontiguousarray(w_out.T).astype(BF16)
    b_outT = np.ascontiguousarray(b_mlp_out.reshape(1, D)).astype(BF16)
    w_in_eff = w_mlp_in * ln2_w[None, :]          # (FF, D)
    def packP(w):
        return np.ascontiguousarray(
            w.reshape(32, 128, 8, 128).transpose(3, 0, 2, 1))
    w_hi = w_in_eff.astype(E5NP).astype(f32)
    w_inP = packP(w_hi).astype(E5NP)
    w_inPb = packP(w_in_eff - w_hi).astype(E5NP)
    w_mlp_outT = np.ascontiguousarray(w_mlp_out.T).astype(BF16)
    b_inT = np.ascontiguousarray(b_mlp_in.reshape(32, 128).T)

    in_maps = []
    for c in range(NCORES):
        h0 = HPC * c
        # x8 keeps fp8e4m3 quantization in the normal range (w ~ 0.02
        # would otherwise be half-subnormal); exp() folds 1/512 back out.
        qrows = w_qkv[h0 * Dh:(h0 + HPC) * Dh] * np.float32(8.0)
        krows = w_qkv[H * Dh + h0 * Dh:H * Dh + (h0 + HPC) * Dh] * np.float32(8.0)
        vrows = w_qkv[2 * H * Dh + h0 * Dh:2 * H * Dh + (h0 + HPC) * Dh] * np.float32(8.0)
        # wq2[kk, p, i, m]: q cols m = iq*64 + hl*32 + po <- row hl*64+iq*32+po
        qperm = np.empty((128, D), f32)   # [m, d]
        kperm = np.empty((128, D), f32)
        for iq in range(2):
            for hl in range(2):
                src = slice(hl * 64 + iq * 32, hl * 64 + iq * 32 + 32)
                dstm = slice(iq * 64 + hl * 32, iq * 64 + hl * 32 + 32)
                qperm[dstm] = qrows[src]
                kperm[dstm] = krows[src]
        wcat = np.concatenate([qperm, kperm, vrows], 0)   # [384 m, 1024 d]
        # -> [kk, p, i, m]
        wq2 = np.ascontiguousarray(
            wcat.T.reshape(4, 2, 128, 384).transpose(0, 2, 1, 3)).astype(E4NP)
        # a2[hl, kt, p, i, q] = alibiT[hl, kt*128+i*64+p, q]
        alibiT = np.ascontiguousarray(
            np.transpose(alibi[0, h0:h0 + HPC], (0, 2, 1)))   # [hl, k, q]
        a2 = np.ascontiguousarray(
            alibiT.reshape(HPC, KT, 2, 64, T).transpose(0, 3, 1, 2, 4)
            * np.float32(256.0)).astype(E4NP)
        x_res = np.ascontiguousarray(x_flat[c * CHUNK:(c + 1) * CHUNK])
        in_maps.append({
            "xT2": xT2, "wq2": wq2, "a2": a2, "w_outT": w_outT,
            "b_outT": b_outT, "x_res": x_res, "w_inP": w_inP,
            "w_inPb": w_inPb, "b_inT": b_inT, "w_mlp_outT": w_mlp_outT,
        })
    return in_maps


def _get_compiled():
    global _COMPILED
    if _COMPILED is None:
        _COMPILED = _build()
    return _COMPILED


def kernel(_trace=False, **inputs):
    nc = _get_compiled()
    in_maps = _host_prep(**inputs)
    res = None
    for attempt in range(3):
        try:
            res = run_bass_kernel_spmd(nc, in_maps,
                                       core_ids=list(range(NCORES)),
                                       trace=_trace)
            break
        except Exception:
            if attempt == 2:
                raise
    out = np.concatenate([res.results[c]["out"] for c in range(NCORES)], 0)
    out = out.reshape(B, T, D).astype(np.float32)
    if _trace:
        return out, res
    return out
